# revision 3
# baseline (speedup 1.0000x reference)
"""Trainium2 Bass kernel v5 for DeformableSincConv1d.

Data parallel: 4 rows/core on 8 cores. fp16 data path.

Key layout trick vs v4: l-columns are ordered u-major (u = l mod 10):
chunk c = 10k+u covers l = u + 10*(128k + p), so the final conv's
stride-10 column walk becomes contiguous k-blocks -> the PE moving
operand streams at full rate (strided rhs was ~6x slower).

dd is [102, 3, 1280]: dd[:, k, 128u + p] = D(u + 10*(128k + p)) on rows
0..50 and D(l+1) on rows 51..101. Final conv rhs for t0 (a = 10t0//51):
dd[0:102, :, 128a:128(a+1)] -- [3, 128] free = s-major contiguous.
"""

import sys

import numpy as np

if "/opt/trn_rl_repo" not in sys.path:
    sys.path.insert(0, "/opt/trn_rl_repo")

SR = 16000
C_OUT = 80
CPAD = 128
K = 51
STRIDE = 10
HALF = (K - 1) // 2

B_FULL = 32
N_CORES = 8
B_LOC = B_FULL // N_CORES
L_FULL = 32000

L_out = (L_FULL - K) // STRIDE + 1            # 3195
T_out = (L_out * K - K) // STRIDE + 1         # 16290
NCH = 30
LPAD = NCH * 128                               # 3840
HWID = LPAD // 2                               # 1920
NPAIR = NCH // 2                               # 15
XLEN = 38448
NSP = 320                                      # valid outputs per t0 (max)
NSPP = 384                                     # computed outputs per t0
CC = 512
NCC = (HWID + CC - 1) // CC                    # 4 chunks: 512,512,512,384


def _host_filters(hz, band):
    hzc = np.clip(hz.astype(np.float32), 0.0, SR / 2).astype(np.float32)
    bandc = np.clip(band.astype(np.float32), 3.0, SR / 2).astype(np.float32)
    t_right = (np.arange(1, HALF + 1, dtype=np.float32) / np.float32(SR)).astype(np.float32)
    low = (hzc - bandc / 2).astype(np.float32)
    high = (hzc + bandc / 2).astype(np.float32)

    def sinc(t):
        ts = np.where(t == 0, np.float32(1.0), t)
        return np.where(t == 0, np.float32(1.0), np.sin(ts) / ts).astype(np.float32)

    a1 = (2 * high).astype(np.float32)
    a2 = (2 * low).astype(np.float32)
    bp_left = (a1 * sinc(a1 * t_right) - a2 * sinc(a2 * t_right)).astype(np.float32)
    bp = np.concatenate([bp_left, np.ones((C_OUT, 1), np.float32), bp_left[:, ::-1]], axis=1)
    return (bp / (2 * bandc)).astype(np.float32)  # [C_OUT, K]


def _host_f102(filt):
    F = np.zeros((102, K, CPAD), np.float32)
    for t0 in range(K):
        a = (STRIDE * t0) // K
        for k2 in range(K):
            kstar = (k2 + STRIDE * t0) % K
            lstar = (STRIDE * t0 + k2) // K
            if lstar == a:
                F[kstar, t0, 0:C_OUT] = filt[:, k2]
            else:
                assert lstar == a + 1
                F[51 + kstar, t0, 0:C_OUT] = filt[:, k2]
    return F.reshape(102, K * CPAD)


def build_program(B_loc=B_LOC, debug=False):
    import concourse.bacc as bacc
    import concourse.tile as tile
    from concourse import bass, mybir

    f32 = mybir.dt.float32
    f16 = mybir.dt.float16
    Alu = mybir.AluOpType

    nc = bacc.Bacc("TRN2", target_bir_lowering=False, debug=debug)

    x_d = nc.dram_tensor("x", [B_loc, 128, 1920], f16, kind="ExternalInput")
    wr22_d = nc.dram_tensor("wr22", [128, 115], f16, kind="ExternalInput")
    lovec_d = nc.dram_tensor("lovec", [115, 1], f32, kind="ExternalInput")
    hivec_d = nc.dram_tensor("hivec", [115, 1], f32, kind="ExternalInput")
    f102_d = nc.dram_tensor("f102", [102, K * CPAD], f16, kind="ExternalInput")
    ident_d = nc.dram_tensor("ident", [128, 128], f16, kind="ExternalInput")
    y_d = nc.dram_tensor("y", [B_loc, C_OUT, K, NSP], f16, kind="ExternalOutput")
    if debug:
        xs1_d = nc.dram_tensor("xs1_dbg", [128, HWID], f16, kind="ExternalOutput")
        q2_d = nc.dram_tensor("q2_dbg", [115, HWID], f16, kind="ExternalOutput")
        dd_d = nc.dram_tensor("dd_dbg", [102, 3, 1280], f16, kind="ExternalOutput")

    xap = x_d[:]
    ONESPAIR = float(np.frombuffer(np.uint32(0x3C003C00).tobytes(), np.float32)[0])

    with tile.TileContext(nc) as tc:
        with (
            tc.tile_pool(name="consts", bufs=1) as consts,
            tc.tile_pool(name="xxa", bufs=1) as xxa,
            tc.tile_pool(name="xxb", bufs=1) as xxb,
            tc.tile_pool(name="xs0a", bufs=1) as xs0a,
            tc.tile_pool(name="xs0b", bufs=1) as xs0b,
            tc.tile_pool(name="xsp", bufs=2) as xsp,
            tc.tile_pool(name="samp", bufs=2) as sampp,
            tc.tile_pool(name="ddp", bufs=2) as ddp,
            tc.tile_pool(name="ysbp", bufs=2) as ysbp,
            tc.tile_pool(name="tpsum", bufs=1, space="PSUM") as tpsum,
            tc.tile_pool(name="opsum", bufs=2, space="PSUM") as opsum,
            tc.tile_pool(name="fpsum", bufs=2, space="PSUM") as fpsum,
        ):
            wr22_sb = consts.tile([128, 115], f16)
            nc.sync.dma_start(out=wr22_sb[:], in_=wr22_d[:])
            lovec_sb = consts.tile([115, 1], f32)
            nc.sync.dma_start(out=lovec_sb[:], in_=lovec_d[:])
            hivec_sb = consts.tile([115, 1], f32)
            nc.sync.dma_start(out=hivec_sb[:], in_=hivec_d[:])
            ident_sb = consts.tile([128, 128], f16)
            nc.sync.dma_start(out=ident_sb[:], in_=ident_d[:])
            f102_sb = consts.tile([102, K * CPAD], f16)
            nc.sync.dma_start(out=f102_sb[:], in_=f102_d[:])

            xx_tiles = []
            for pool in (xxa, xxb):
                t = pool.tile([128, 3, 10, 64], f16)
                xx_tiles.append(t)
            xs0_tiles = []
            for pool in (xs0a, xs0b):
                t = pool.tile([128, HWID], f16)
                nc.vector.memset(t[0:1, :].bitcast(f32), 0.0)
                nc.vector.memset(t[64:65, :].bitcast(f32), 0.0)
                xs0_tiles.append(t)

            def emit_load(b):
                # host pre-arranged: xr[b, p, 1920] = im2col rows incl ones/pad
                xx = xx_tiles[b % 2]
                nc.scalar.dma_start(out=xx[:], in_=x_d[b])
                return xx

            def _ku(c):
                return c // 10, c % 10

            def emit_front(b, xx):
                xs1 = xsp.tile([128, HWID], f16, tag="xs1")
                for (c_lo, c_n) in ((0, 8), (8, 7)):
                    ptl = tpsum.tile([64, 1024], f16, tag="tpL")
                    ptu = tpsum.tile([64, 1024], f16, tag="tpU")
                    for i in range(c_n):
                        kl, ul = _ku(c_lo + i)
                        ku_, uu = _ku(NPAIR + c_lo + i)
                        nc.tensor.transpose(
                            ptl[:, i * 128:(i + 1) * 128],
                            xx[:, kl, ul, :], ident_sb[:])
                        nc.tensor.transpose(
                            ptu[:, i * 128:(i + 1) * 128],
                            xx[:, ku_, uu, :], ident_sb[:])
                    n = c_n * 128
                    sl = slice(c_lo * 128, c_lo * 128 + n)
                    nc.scalar.copy(xs1[0:64, sl], ptl[:, :n])
                    nc.vector.tensor_copy(xs1[64:128, sl], ptu[:, :n])

                xs0 = xs0_tiles[b % 2]
                nc.sync.dma_start(out=xs0[1:52], in_=xs1[0:51])
                nc.sync.dma_start(out=xs0[65:116], in_=xs1[64:115])
                xs2 = xsp.tile([128, HWID], f16, tag="xs2")
                nc.sync.dma_start(out=xs2[0:51], in_=xs1[1:52])
                nc.sync.dma_start(out=xs2[64:115], in_=xs1[65:116])

                q2 = sampp.tile([115, HWID], f16, tag="q2")
                for c7 in range(NCC):
                    n = min(CC, HWID - c7 * CC)
                    sl = slice(c7 * CC, c7 * CC + n)
                    po = opsum.tile([115, CC], f32, tag="po")
                    nc.tensor.matmul(po[:, :n], wr22_sb[:], xs1[:, sl],
                                     start=True, stop=True)
                    nc.vector.tensor_scalar(q2[:, sl], po[:, :n], lovec_sb[:],
                                            hivec_sb[:], op0=Alu.max, op1=Alu.min)

                ef = sampp.tile([115, HWID], f16, tag="ef")
                nc.vector.tensor_sub(ef[:], xs2[0:115], xs1[0:115])
                eb = sampp.tile([115, HWID], f16, tag="eb")
                nc.vector.tensor_sub(eb[:], xs1[0:115], xs0[0:115])
                qp = sampp.tile([115, HWID], f16, tag="qp")
                nc.vector.tensor_scalar(qp[:], q2[:], 0.0, None, op0=Alu.max)
                qm = sampp.tile([115, HWID], f16, tag="qm")
                nc.vector.tensor_scalar(qm[:], q2[:], 0.0, None, op0=Alu.min)
                t1 = sampp.tile([115, HWID], f16, tag="t1")
                nc.vector.tensor_mul(t1[:], qp[:], ef[:])
                t2 = sampp.tile([115, HWID], f16, tag="t2")
                nc.vector.tensor_mul(t2[:], qm[:], eb[:])
                s2 = sampp.tile([115, HWID], f16, tag="s2")
                nc.vector.tensor_add(s2[:], t1[:], t2[:])

                # dd[:, k, 128u+p] = D(u + 10*(128k+p)); chunk c=10k+u sits at
                # [k, 128u:128u+128]. s2/xs1 columns are chunk-pair order:
                # lower cols 128c'+p -> chunk c' (c'=0..14), upper -> c'+15.
                dd = ddp.tile([102, 3, 1280], f16, tag="dd")
                nc.vector.tensor_add(dd[0:51, 0, :], s2[0:51, 0:1280],
                                     xs1[0:51, 0:1280])
                nc.vector.tensor_add(dd[0:51, 1, 0:640], s2[0:51, 1280:1920],
                                     xs1[0:51, 1280:1920])
                nc.vector.tensor_add(dd[0:51, 1, 640:1280], s2[64:115, 0:640],
                                     xs1[64:115, 0:640])
                nc.vector.tensor_add(dd[0:51, 2, :], s2[64:115, 640:1920],
                                     xs1[64:115, 640:1920])
                # 102-stack shift: D(l+1). u<=8: chunk (k,u+1) same p -> col+128.
                # u=9: chunk (k,0) p+1 -> k-block cols 1..128.
                nc.sync.dma_start(out=dd[51:102, :, 0:1152],
                                  in_=dd[0:51, :, 128:1280])
                nc.sync.dma_start(out=dd[51:102, :, 1152:1280],
                                  in_=dd[0:51, :, 1:129])
                # u=9, p=127 wraps to the next k-block: D(10*128*(k+1))
                nc.sync.dma_start(out=dd[51:102, 0:2, 1279:1280],
                                  in_=dd[0:51, 1:3, 0:1])
                if debug and b == 0:
                    nc.sync.dma_start(out=xs1_d[:], in_=xs1[:])
                    nc.sync.dma_start(out=q2_d[:], in_=q2[:])
                    nc.sync.dma_start(out=dd_d[:], in_=dd[:])
                return dd

            def emit_final(b, dd):
                ysb = ysbp.tile([C_OUT, K, NSP], f16, tag="ysb")
                npairs = (K + 1) // 2
                for pi in range(npairs):
                    t0a = 2 * pi
                    nt = min(2, K - t0a)
                    fp = fpsum.tile([CPAD, 2, 512], f32, tag="fp")
                    for j in range(nt):
                        t0 = t0a + j
                        a = (STRIDE * t0) // K
                        rhs = dd[0:102, :, 128 * a:128 * (a + 1)]
                        lhsT = f102_sb[0:102, t0 * CPAD:(t0 + 1) * CPAD]
                        nc.tensor.matmul(fp[:, j, 0:NSPP], lhsT, rhs,
                                         start=True, stop=True)
                    dst = ysb[:, t0a:t0a + nt, :]
                    src_ap = fp[0:C_OUT, 0:nt, 0:NSP]
                    if pi % 4 == 3:
                        nc.vector.tensor_copy(dst, src_ap)
                    else:
                        nc.scalar.copy(dst, src_ap)
                    if t0a == 16:
                        nc.sync.dma_start(out=y_d[b, :, 0:16], in_=ysb[:, 0:16, :])
                    elif t0a == 34:
                        nc.sync.dma_start(out=y_d[b, :, 16:34], in_=ysb[:, 16:34, :])
                nc.sync.dma_start(out=y_d[b, :, 34:K], in_=ysb[:, 34:K, :])

            emit_load(0)
            emit_load(1)
            dds = {}
            dds[0] = emit_front(0, xx_tiles[0])
            emit_load(2)
            dds[1] = emit_front(1, xx_tiles[1])
            emit_final(0, dds[0])
            emit_load(3)
            dds[2] = emit_front(2, xx_tiles[0])
            emit_final(1, dds[1])
            dds[3] = emit_front(3, xx_tiles[1])
            emit_final(2, dds[2])
            emit_final(3, dds[3])

    nc.compile()
    return nc


def _host_inputs(x, hz, band, offset_w, offset_b, B_loc=B_LOC):
    filt = _host_filters(hz, band)
    f102 = np.ascontiguousarray(_host_f102(filt).astype(np.float16))

    # wr2p rows are XS1 taps: XS1[j'] = x[10l + j']; rows 54/55 = ones
    wr2p = np.zeros((64, K), np.float32)
    wr2p[0:51, :] = offset_w[:, 0, :].T
    wr2p[54, :] = offset_b
    wr22 = np.zeros((128, 115), np.float32)
    wr22[0:64, 0:51] = wr2p
    wr22[64:128, 64:115] = wr2p
    wr22 = np.ascontiguousarray(wr22.astype(np.float16))

    ks = np.arange(K, dtype=np.float32)
    lovec = np.zeros((115, 1), np.float32)
    hivec = np.zeros((115, 1), np.float32)
    lovec[0:51, 0] = -ks
    hivec[0:51, 0] = 50 - ks
    lovec[64:115, 0] = -ks
    hivec[64:115, 0] = 50 - ks
    ident = np.ascontiguousarray(np.eye(128, dtype=np.float16))

    B = x.shape[0]
    xpad = np.zeros((B, XLEN), np.float32)
    xpad[:, 1:1 + L_FULL] = x
    p_ = np.arange(128); kk = np.arange(3); uu = np.arange(10); jj = np.arange(53)
    idx = (1 + 100 * p_[:, None, None, None] + 12800 * kk[None, :, None, None]
           + 10 * uu[None, None, :, None] + jj[None, None, None, :])
    xr = np.zeros((B, 128, 3, 10, 64), np.float32)
    xr[:, :, :, :, 0:53] = xpad[:, idx]
    xr[:, :, :, :, 54:56] = 1.0
    xr = xr.reshape(B, 128, 1920).astype(np.float16)

    n_cores = B // B_loc
    in_maps = []
    for i in range(n_cores):
        in_maps.append({
            "x": np.ascontiguousarray(xr[i * B_loc:(i + 1) * B_loc]),
            "wr22": wr22,
            "lovec": lovec,
            "hivec": hivec,
            "f102": f102,
            "ident": ident,
        })
    return in_maps


def _host_assemble(outs):
    """outs: per-core y [B_loc, C_OUT, K, NSP] f16 -> full [B, C, T_out] f32."""
    ydev = np.concatenate(outs, axis=0).astype(np.float32)  # [B, C, K, NSP]
    B = ydev.shape[0]
    y = np.empty((B, C_OUT, T_out), np.float32)
    for t0 in range(K):
        ns = (T_out - t0 + K - 1) // K
        y[:, :, t0::K] = ydev[:, :, t0, :ns]
    return y


_CACHED = {}


def _get_program():
    key = B_LOC
    if key not in _CACHED:
        _CACHED[key] = build_program(B_LOC)
    return _CACHED[key]


def kernel(x, hz, band, offset_w, offset_b):
    from concourse.bass_utils import run_bass_kernel_spmd

    x = np.asarray(x, dtype=np.float32)
    hz = np.asarray(hz, dtype=np.float32)
    band = np.asarray(band, dtype=np.float32)
    offset_w = np.asarray(offset_w, dtype=np.float32)
    offset_b = np.asarray(offset_b, dtype=np.float32)

    nc = _get_program()
    in_maps = _host_inputs(x, hz, band, offset_w, offset_b, B_LOC)
    res = run_bass_kernel_spmd(nc, in_maps, list(range(N_CORES)))
    outs = [res.results[i]["y"] for i in range(N_CORES)]
    return _host_assemble(outs)


# revision 4
# speedup vs baseline: 1.0026x; 1.0026x over previous
"""Trainium2 Bass kernel v5 for DeformableSincConv1d.

Data parallel: 4 rows/core on 8 cores. fp16 data path.

Key layout trick vs v4: l-columns are ordered u-major (u = l mod 10):
chunk c = 10k+u covers l = u + 10*(128k + p), so the final conv's
stride-10 column walk becomes contiguous k-blocks -> the PE moving
operand streams at full rate (strided rhs was ~6x slower).

dd is [102, 3, 1280]: dd[:, k, 128u + p] = D(u + 10*(128k + p)) on rows
0..50 and D(l+1) on rows 51..101. Final conv rhs for t0 (a = 10t0//51):
dd[0:102, :, 128a:128(a+1)] -- [3, 128] free = s-major contiguous.
"""

import sys

import numpy as np

if "/opt/trn_rl_repo" not in sys.path:
    sys.path.insert(0, "/opt/trn_rl_repo")

SR = 16000
C_OUT = 80
CPAD = 128
K = 51
STRIDE = 10
HALF = (K - 1) // 2

B_FULL = 32
N_CORES = 8
B_LOC = B_FULL // N_CORES
L_FULL = 32000

L_out = (L_FULL - K) // STRIDE + 1            # 3195
T_out = (L_out * K - K) // STRIDE + 1         # 16290
NCH = 30
LPAD = NCH * 128                               # 3840
HWID = LPAD // 2                               # 1920
NPAIR = NCH // 2                               # 15
XLEN = 38448
NSP = 320                                      # valid outputs per t0 (max)
NSPP = 384                                     # computed outputs per t0
CC = 512
NCC = (HWID + CC - 1) // CC                    # 4 chunks: 512,512,512,384


def _host_filters(hz, band):
    hzc = np.clip(hz.astype(np.float32), 0.0, SR / 2).astype(np.float32)
    bandc = np.clip(band.astype(np.float32), 3.0, SR / 2).astype(np.float32)
    t_right = (np.arange(1, HALF + 1, dtype=np.float32) / np.float32(SR)).astype(np.float32)
    low = (hzc - bandc / 2).astype(np.float32)
    high = (hzc + bandc / 2).astype(np.float32)

    def sinc(t):
        ts = np.where(t == 0, np.float32(1.0), t)
        return np.where(t == 0, np.float32(1.0), np.sin(ts) / ts).astype(np.float32)

    a1 = (2 * high).astype(np.float32)
    a2 = (2 * low).astype(np.float32)
    bp_left = (a1 * sinc(a1 * t_right) - a2 * sinc(a2 * t_right)).astype(np.float32)
    bp = np.concatenate([bp_left, np.ones((C_OUT, 1), np.float32), bp_left[:, ::-1]], axis=1)
    return (bp / (2 * bandc)).astype(np.float32)  # [C_OUT, K]


def _host_f102(filt):
    F = np.zeros((102, K, CPAD), np.float32)
    for t0 in range(K):
        a = (STRIDE * t0) // K
        for k2 in range(K):
            kstar = (k2 + STRIDE * t0) % K
            lstar = (STRIDE * t0 + k2) // K
            if lstar == a:
                F[kstar, t0, 0:C_OUT] = filt[:, k2]
            else:
                assert lstar == a + 1
                F[51 + kstar, t0, 0:C_OUT] = filt[:, k2]
    return F.reshape(102, K * CPAD)


def build_program(B_loc=B_LOC, debug=False):
    import concourse.bacc as bacc
    import concourse.tile as tile
    from concourse import bass, mybir

    f32 = mybir.dt.float32
    f16 = mybir.dt.float16
    Alu = mybir.AluOpType

    nc = bacc.Bacc("TRN2", target_bir_lowering=False, debug=debug)

    x_d = nc.dram_tensor("x", [B_loc, 128, 1920], f16, kind="ExternalInput")
    wr22_d = nc.dram_tensor("wr22", [128, 115], f16, kind="ExternalInput")
    lovec_d = nc.dram_tensor("lovec", [115, 1], f32, kind="ExternalInput")
    hivec_d = nc.dram_tensor("hivec", [115, 1], f32, kind="ExternalInput")
    f102_d = nc.dram_tensor("f102", [102, K * CPAD], f16, kind="ExternalInput")
    ident_d = nc.dram_tensor("ident", [128, 128], f16, kind="ExternalInput")
    y_d = nc.dram_tensor("y", [B_loc, C_OUT, K, NSP], f16, kind="ExternalOutput")
    if debug:
        xs1_d = nc.dram_tensor("xs1_dbg", [128, HWID], f16, kind="ExternalOutput")
        q2_d = nc.dram_tensor("q2_dbg", [115, HWID], f16, kind="ExternalOutput")
        dd_d = nc.dram_tensor("dd_dbg", [102, 3, 1280], f16, kind="ExternalOutput")

    xap = x_d[:]
    ONESPAIR = float(np.frombuffer(np.uint32(0x3C003C00).tobytes(), np.float32)[0])

    with tile.TileContext(nc) as tc:
        with (
            tc.tile_pool(name="consts", bufs=1) as consts,
            tc.tile_pool(name="xxa", bufs=1) as xxa,
            tc.tile_pool(name="xxb", bufs=1) as xxb,
            tc.tile_pool(name="xs0a", bufs=1) as xs0a,
            tc.tile_pool(name="xs0b", bufs=1) as xs0b,
            tc.tile_pool(name="xsp", bufs=2) as xsp,
            tc.tile_pool(name="samp", bufs=2) as sampp,
            tc.tile_pool(name="ddp", bufs=2) as ddp,
            tc.tile_pool(name="ysbp", bufs=2) as ysbp,
            tc.tile_pool(name="tpsum", bufs=1, space="PSUM") as tpsum,
            tc.tile_pool(name="opsum", bufs=2, space="PSUM") as opsum,
            tc.tile_pool(name="fpsum", bufs=2, space="PSUM") as fpsum,
        ):
            wr22_sb = consts.tile([128, 115], f16)
            nc.sync.dma_start(out=wr22_sb[:], in_=wr22_d[:])
            lovec_sb = consts.tile([115, 1], f32)
            nc.sync.dma_start(out=lovec_sb[:], in_=lovec_d[:])
            hivec_sb = consts.tile([115, 1], f32)
            nc.sync.dma_start(out=hivec_sb[:], in_=hivec_d[:])
            ident_sb = consts.tile([128, 128], f16)
            nc.sync.dma_start(out=ident_sb[:], in_=ident_d[:])
            f102_sb = consts.tile([102, K * CPAD], f16)
            nc.sync.dma_start(out=f102_sb[:], in_=f102_d[:])

            xx_tiles = []
            for pool in (xxa, xxb):
                t = pool.tile([128, 3, 10, 64], f16)
                xx_tiles.append(t)
            xs0_tiles = []
            for pool in (xs0a, xs0b):
                t = pool.tile([128, HWID], f16)
                nc.vector.memset(t[0:1, :].bitcast(f32), 0.0)
                nc.vector.memset(t[64:65, :].bitcast(f32), 0.0)
                xs0_tiles.append(t)

            def emit_load(b):
                # host pre-arranged: xr[b, p, 1920] = im2col rows incl ones/pad
                xx = xx_tiles[b % 2]
                nc.scalar.dma_start(out=xx[:], in_=x_d[b])
                return xx

            def _ku(c):
                return c // 10, c % 10

            def emit_front(b, xx):
                xs1 = xsp.tile([128, HWID], f16, tag="xs1")
                for (c_lo, c_n) in ((0, 8), (8, 7)):
                    ptl = tpsum.tile([64, 1024], f16, tag="tpL")
                    ptu = tpsum.tile([64, 1024], f16, tag="tpU")
                    for i in range(c_n):
                        kl, ul = _ku(c_lo + i)
                        ku_, uu = _ku(NPAIR + c_lo + i)
                        nc.tensor.transpose(
                            ptl[:, i * 128:(i + 1) * 128],
                            xx[:, kl, ul, :], ident_sb[:])
                        nc.tensor.transpose(
                            ptu[:, i * 128:(i + 1) * 128],
                            xx[:, ku_, uu, :], ident_sb[:])
                    n = c_n * 128
                    sl = slice(c_lo * 128, c_lo * 128 + n)
                    nc.scalar.copy(xs1[0:64, sl], ptl[:, :n])
                    nc.vector.tensor_copy(xs1[64:128, sl], ptu[:, :n])

                xs0 = xs0_tiles[b % 2]
                nc.gpsimd.dma_start(out=xs0[1:52], in_=xs1[0:51])
                nc.gpsimd.dma_start(out=xs0[65:116], in_=xs1[64:115])
                xs2 = xsp.tile([128, HWID], f16, tag="xs2")
                nc.gpsimd.dma_start(out=xs2[0:51], in_=xs1[1:52])
                nc.gpsimd.dma_start(out=xs2[64:115], in_=xs1[65:116])

                q2 = sampp.tile([115, HWID], f16, tag="q2")
                for c7 in range(NCC):
                    n = min(CC, HWID - c7 * CC)
                    sl = slice(c7 * CC, c7 * CC + n)
                    po = opsum.tile([115, CC], f32, tag="po")
                    nc.tensor.matmul(po[:, :n], wr22_sb[:], xs1[:, sl],
                                     start=True, stop=True)
                    nc.vector.tensor_scalar(q2[:, sl], po[:, :n], lovec_sb[:],
                                            hivec_sb[:], op0=Alu.max, op1=Alu.min)

                ef = sampp.tile([115, HWID], f16, tag="ef")
                nc.vector.tensor_sub(ef[:], xs2[0:115], xs1[0:115])
                eb = sampp.tile([115, HWID], f16, tag="eb")
                nc.vector.tensor_sub(eb[:], xs1[0:115], xs0[0:115])
                qp = sampp.tile([115, HWID], f16, tag="qp")
                nc.vector.tensor_scalar(qp[:], q2[:], 0.0, None, op0=Alu.max)
                qm = sampp.tile([115, HWID], f16, tag="qm")
                nc.vector.tensor_scalar(qm[:], q2[:], 0.0, None, op0=Alu.min)
                t1 = sampp.tile([115, HWID], f16, tag="t1")
                nc.vector.tensor_mul(t1[:], qp[:], ef[:])
                t2 = sampp.tile([115, HWID], f16, tag="t2")
                nc.vector.tensor_mul(t2[:], qm[:], eb[:])
                s2 = sampp.tile([115, HWID], f16, tag="s2")
                nc.vector.tensor_add(s2[:], t1[:], t2[:])

                # dd[:, k, 128u+p] = D(u + 10*(128k+p)); chunk c=10k+u sits at
                # [k, 128u:128u+128]. s2/xs1 columns are chunk-pair order:
                # lower cols 128c'+p -> chunk c' (c'=0..14), upper -> c'+15.
                dd = ddp.tile([102, 3, 1280], f16, tag="dd")
                nc.vector.tensor_add(dd[0:51, 0, :], s2[0:51, 0:1280],
                                     xs1[0:51, 0:1280])
                nc.vector.tensor_add(dd[0:51, 1, 0:640], s2[0:51, 1280:1920],
                                     xs1[0:51, 1280:1920])
                nc.vector.tensor_add(dd[0:51, 1, 640:1280], s2[64:115, 0:640],
                                     xs1[64:115, 0:640])
                nc.vector.tensor_add(dd[0:51, 2, :], s2[64:115, 640:1920],
                                     xs1[64:115, 640:1920])
                # 102-stack shift: D(l+1). u<=8: chunk (k,u+1) same p -> col+128.
                # u=9: chunk (k,0) p+1 -> k-block cols 1..128.
                nc.sync.dma_start(out=dd[51:102, :, 0:1152],
                                  in_=dd[0:51, :, 128:1280])
                nc.sync.dma_start(out=dd[51:102, :, 1152:1280],
                                  in_=dd[0:51, :, 1:129])
                # u=9, p=127 wraps to the next k-block: D(10*128*(k+1))
                nc.sync.dma_start(out=dd[51:102, 0:2, 1279:1280],
                                  in_=dd[0:51, 1:3, 0:1])
                if debug and b == 0:
                    nc.sync.dma_start(out=xs1_d[:], in_=xs1[:])
                    nc.sync.dma_start(out=q2_d[:], in_=q2[:])
                    nc.sync.dma_start(out=dd_d[:], in_=dd[:])
                return dd

            def emit_final(b, dd):
                ysb = ysbp.tile([C_OUT, K, NSP], f16, tag="ysb")
                npairs = (K + 1) // 2
                for pi in range(npairs):
                    t0a = 2 * pi
                    nt = min(2, K - t0a)
                    fp = fpsum.tile([CPAD, 2, 512], f32, tag="fp")
                    for j in range(nt):
                        t0 = t0a + j
                        a = (STRIDE * t0) // K
                        rhs = dd[0:102, :, 128 * a:128 * (a + 1)]
                        lhsT = f102_sb[0:102, t0 * CPAD:(t0 + 1) * CPAD]
                        nc.tensor.matmul(fp[:, j, 0:NSPP], lhsT, rhs,
                                         start=True, stop=True)
                    dst = ysb[:, t0a:t0a + nt, :]
                    src_ap = fp[0:C_OUT, 0:nt, 0:NSP]
                    if pi % 4 == 3:
                        nc.vector.tensor_copy(dst, src_ap)
                    else:
                        nc.scalar.copy(dst, src_ap)
                    if t0a in (8, 16, 24, 34, 42):
                        lo = {8: 0, 16: 8, 24: 16, 34: 24, 42: 34}[t0a]
                        nc.sync.dma_start(out=y_d[b, :, lo:t0a],
                                          in_=ysb[:, lo:t0a, :])
                nc.sync.dma_start(out=y_d[b, :, 42:K], in_=ysb[:, 42:K, :])

            emit_load(0)
            emit_load(1)
            dds = {}
            dds[0] = emit_front(0, xx_tiles[0])
            emit_load(2)
            dds[1] = emit_front(1, xx_tiles[1])
            emit_final(0, dds[0])
            emit_load(3)
            dds[2] = emit_front(2, xx_tiles[0])
            emit_final(1, dds[1])
            dds[3] = emit_front(3, xx_tiles[1])
            emit_final(2, dds[2])
            emit_final(3, dds[3])

    nc.compile()
    return nc


def _host_inputs(x, hz, band, offset_w, offset_b, B_loc=B_LOC):
    filt = _host_filters(hz, band)
    f102 = np.ascontiguousarray(_host_f102(filt).astype(np.float16))

    # wr2p rows are XS1 taps: XS1[j'] = x[10l + j']; rows 54/55 = ones
    wr2p = np.zeros((64, K), np.float32)
    wr2p[0:51, :] = offset_w[:, 0, :].T
    wr2p[54, :] = offset_b
    wr22 = np.zeros((128, 115), np.float32)
    wr22[0:64, 0:51] = wr2p
    wr22[64:128, 64:115] = wr2p
    wr22 = np.ascontiguousarray(wr22.astype(np.float16))

    ks = np.arange(K, dtype=np.float32)
    lovec = np.zeros((115, 1), np.float32)
    hivec = np.zeros((115, 1), np.float32)
    lovec[0:51, 0] = -ks
    hivec[0:51, 0] = 50 - ks
    lovec[64:115, 0] = -ks
    hivec[64:115, 0] = 50 - ks
    ident = np.ascontiguousarray(np.eye(128, dtype=np.float16))

    B = x.shape[0]
    xpad = np.zeros((B, XLEN), np.float32)
    xpad[:, 1:1 + L_FULL] = x
    p_ = np.arange(128); kk = np.arange(3); uu = np.arange(10); jj = np.arange(53)
    idx = (1 + 100 * p_[:, None, None, None] + 12800 * kk[None, :, None, None]
           + 10 * uu[None, None, :, None] + jj[None, None, None, :])
    xr = np.zeros((B, 128, 3, 10, 64), np.float32)
    xr[:, :, :, :, 0:53] = xpad[:, idx]
    xr[:, :, :, :, 54:56] = 1.0
    xr = xr.reshape(B, 128, 1920).astype(np.float16)

    n_cores = B // B_loc
    in_maps = []
    for i in range(n_cores):
        in_maps.append({
            "x": np.ascontiguousarray(xr[i * B_loc:(i + 1) * B_loc]),
            "wr22": wr22,
            "lovec": lovec,
            "hivec": hivec,
            "f102": f102,
            "ident": ident,
        })
    return in_maps


def _host_assemble(outs):
    """outs: per-core y [B_loc, C_OUT, K, NSP] f16 -> full [B, C, T_out] f32."""
    ydev = np.concatenate(outs, axis=0).astype(np.float32)  # [B, C, K, NSP]
    B = ydev.shape[0]
    y = np.empty((B, C_OUT, T_out), np.float32)
    for t0 in range(K):
        ns = (T_out - t0 + K - 1) // K
        y[:, :, t0::K] = ydev[:, :, t0, :ns]
    return y


_CACHED = {}


def _get_program():
    key = B_LOC
    if key not in _CACHED:
        _CACHED[key] = build_program(B_LOC)
    return _CACHED[key]


def kernel(x, hz, band, offset_w, offset_b):
    from concourse.bass_utils import run_bass_kernel_spmd

    x = np.asarray(x, dtype=np.float32)
    hz = np.asarray(hz, dtype=np.float32)
    band = np.asarray(band, dtype=np.float32)
    offset_w = np.asarray(offset_w, dtype=np.float32)
    offset_b = np.asarray(offset_b, dtype=np.float32)

    nc = _get_program()
    in_maps = _host_inputs(x, hz, band, offset_w, offset_b, B_LOC)
    res = run_bass_kernel_spmd(nc, in_maps, list(range(N_CORES)))
    outs = [res.results[i]["y"] for i in range(N_CORES)]
    return _host_assemble(outs)


# revision 5
# speedup vs baseline: 1.0140x; 1.0114x over previous
"""Trainium2 Bass kernel v5 for DeformableSincConv1d.

Data parallel: 4 rows/core on 8 cores. fp16 data path.

Key layout trick vs v4: l-columns are ordered u-major (u = l mod 10):
chunk c = 10k+u covers l = u + 10*(128k + p), so the final conv's
stride-10 column walk becomes contiguous k-blocks -> the PE moving
operand streams at full rate (strided rhs was ~6x slower).

dd is [102, 3, 1280]: dd[:, k, 128u + p] = D(u + 10*(128k + p)) on rows
0..50 and D(l+1) on rows 51..101. Final conv rhs for t0 (a = 10t0//51):
dd[0:102, :, 128a:128(a+1)] -- [3, 128] free = s-major contiguous.
"""

import sys

import numpy as np

if "/opt/trn_rl_repo" not in sys.path:
    sys.path.insert(0, "/opt/trn_rl_repo")

SR = 16000
C_OUT = 80
CPAD = 128
K = 51
STRIDE = 10
HALF = (K - 1) // 2

B_FULL = 32
N_CORES = 8
B_LOC = B_FULL // N_CORES
L_FULL = 32000

L_out = (L_FULL - K) // STRIDE + 1            # 3195
T_out = (L_out * K - K) // STRIDE + 1         # 16290
NCH = 30
LPAD = NCH * 128                               # 3840
HWID = LPAD // 2                               # 1920
NPAIR = NCH // 2                               # 15
XLEN = 38448
NSP = 320                                      # valid outputs per t0 (max)
NSPP = 384                                     # computed outputs per t0
CC = 512
NCC = (HWID + CC - 1) // CC                    # 4 chunks: 512,512,512,384


def _host_filters(hz, band):
    hzc = np.clip(hz.astype(np.float32), 0.0, SR / 2).astype(np.float32)
    bandc = np.clip(band.astype(np.float32), 3.0, SR / 2).astype(np.float32)
    t_right = (np.arange(1, HALF + 1, dtype=np.float32) / np.float32(SR)).astype(np.float32)
    low = (hzc - bandc / 2).astype(np.float32)
    high = (hzc + bandc / 2).astype(np.float32)

    def sinc(t):
        ts = np.where(t == 0, np.float32(1.0), t)
        return np.where(t == 0, np.float32(1.0), np.sin(ts) / ts).astype(np.float32)

    a1 = (2 * high).astype(np.float32)
    a2 = (2 * low).astype(np.float32)
    bp_left = (a1 * sinc(a1 * t_right) - a2 * sinc(a2 * t_right)).astype(np.float32)
    bp = np.concatenate([bp_left, np.ones((C_OUT, 1), np.float32), bp_left[:, ::-1]], axis=1)
    return (bp / (2 * bandc)).astype(np.float32)  # [C_OUT, K]


def _host_f102(filt):
    F = np.zeros((102, K, CPAD), np.float32)
    for t0 in range(K):
        a = (STRIDE * t0) // K
        for k2 in range(K):
            kstar = (k2 + STRIDE * t0) % K
            lstar = (STRIDE * t0 + k2) // K
            if lstar == a:
                F[kstar, t0, 0:C_OUT] = filt[:, k2]
            else:
                assert lstar == a + 1
                F[51 + kstar, t0, 0:C_OUT] = filt[:, k2]
    return F.reshape(102, K * CPAD)


def build_program(B_loc=B_LOC, debug=False):
    import concourse.bacc as bacc
    import concourse.tile as tile
    from concourse import bass, mybir

    f32 = mybir.dt.float32
    f16 = mybir.dt.float16
    Alu = mybir.AluOpType

    nc = bacc.Bacc("TRN2", target_bir_lowering=False, debug=debug)

    x_d = nc.dram_tensor("x", [B_loc, 128, 1920], f16, kind="ExternalInput")
    wr22_d = nc.dram_tensor("wr22", [128, 115], f16, kind="ExternalInput")
    lovec_d = nc.dram_tensor("lovec", [115, 1], f32, kind="ExternalInput")
    hivec_d = nc.dram_tensor("hivec", [115, 1], f32, kind="ExternalInput")
    f102_d = nc.dram_tensor("f102", [102, K * CPAD], f16, kind="ExternalInput")
    ident_d = nc.dram_tensor("ident", [128, 128], f16, kind="ExternalInput")
    y_d = nc.dram_tensor("y", [B_loc, C_OUT, K, NSP], f16, kind="ExternalOutput")
    if debug:
        xs1_d = nc.dram_tensor("xs1_dbg", [128, HWID], f16, kind="ExternalOutput")
        q2_d = nc.dram_tensor("q2_dbg", [115, HWID], f16, kind="ExternalOutput")
        dd_d = nc.dram_tensor("dd_dbg", [102, 3, 1280], f16, kind="ExternalOutput")

    xap = x_d[:]
    ONESPAIR = float(np.frombuffer(np.uint32(0x3C003C00).tobytes(), np.float32)[0])

    with tile.TileContext(nc) as tc:
        with (
            tc.tile_pool(name="consts", bufs=1) as consts,
            tc.tile_pool(name="xxa", bufs=1) as xxa,
            tc.tile_pool(name="xxb", bufs=1) as xxb,
            tc.tile_pool(name="xs0a", bufs=1) as xs0a,
            tc.tile_pool(name="xs0b", bufs=1) as xs0b,
            tc.tile_pool(name="xsp", bufs=2) as xsp,
            tc.tile_pool(name="samp", bufs=2) as sampp,
            tc.tile_pool(name="ddp", bufs=2) as ddp,
            tc.tile_pool(name="ysbp", bufs=2) as ysbp,
            tc.tile_pool(name="tpsum", bufs=1, space="PSUM") as tpsum,
            tc.tile_pool(name="opsum", bufs=2, space="PSUM") as opsum,
            tc.tile_pool(name="fpsum", bufs=4, space="PSUM") as fpsum,
        ):
            wr22_sb = consts.tile([128, 115], f16)
            nc.sync.dma_start(out=wr22_sb[:], in_=wr22_d[:])
            lovec_sb = consts.tile([115, 1], f32)
            nc.sync.dma_start(out=lovec_sb[:], in_=lovec_d[:])
            hivec_sb = consts.tile([115, 1], f32)
            nc.sync.dma_start(out=hivec_sb[:], in_=hivec_d[:])
            ident_sb = consts.tile([128, 128], f16)
            nc.sync.dma_start(out=ident_sb[:], in_=ident_d[:])
            f102_sb = consts.tile([102, K * CPAD], f16)
            nc.sync.dma_start(out=f102_sb[:], in_=f102_d[:])

            xx_tiles = []
            for pool in (xxa, xxb):
                t = pool.tile([128, 3, 10, 64], f16)
                xx_tiles.append(t)
            xs0_tiles = []
            for pool in (xs0a, xs0b):
                t = pool.tile([128, HWID], f16)
                nc.vector.memset(t[0:1, :].bitcast(f32), 0.0)
                nc.vector.memset(t[64:65, :].bitcast(f32), 0.0)
                xs0_tiles.append(t)

            def emit_load(b):
                # host pre-arranged: xr[b, p, 1920] = im2col rows incl ones/pad
                xx = xx_tiles[b % 2]
                nc.scalar.dma_start(out=xx[:], in_=x_d[b])
                return xx

            def _ku(c):
                return c // 10, c % 10

            def emit_front(b, xx):
                xs1 = xsp.tile([128, HWID], f16, tag="xs1")
                for (c_lo, c_n) in ((0, 8), (8, 7)):
                    ptl = tpsum.tile([64, 1024], f16, tag="tpL")
                    ptu = tpsum.tile([64, 1024], f16, tag="tpU")
                    for i in range(c_n):
                        kl, ul = _ku(c_lo + i)
                        ku_, uu = _ku(NPAIR + c_lo + i)
                        nc.tensor.transpose(
                            ptl[:, i * 128:(i + 1) * 128],
                            xx[:, kl, ul, :], ident_sb[:])
                        nc.tensor.transpose(
                            ptu[:, i * 128:(i + 1) * 128],
                            xx[:, ku_, uu, :], ident_sb[:])
                    n = c_n * 128
                    sl = slice(c_lo * 128, c_lo * 128 + n)
                    nc.scalar.copy(xs1[0:64, sl], ptl[:, :n])
                    nc.vector.tensor_copy(xs1[64:128, sl], ptu[:, :n])

                xs0 = xs0_tiles[b % 2]
                nc.gpsimd.dma_start(out=xs0[1:52], in_=xs1[0:51])
                nc.gpsimd.dma_start(out=xs0[65:116], in_=xs1[64:115])
                xs2 = xsp.tile([128, HWID], f16, tag="xs2")
                nc.gpsimd.dma_start(out=xs2[0:51], in_=xs1[1:52])
                nc.gpsimd.dma_start(out=xs2[64:115], in_=xs1[65:116])

                q2 = sampp.tile([115, HWID], f16, tag="q2")
                for c7 in range(NCC):
                    n = min(CC, HWID - c7 * CC)
                    sl = slice(c7 * CC, c7 * CC + n)
                    po = opsum.tile([115, CC], f32, tag="po")
                    nc.tensor.matmul(po[:, :n], wr22_sb[:], xs1[:, sl],
                                     start=True, stop=True)
                    nc.vector.tensor_scalar(q2[:, sl], po[:, :n], lovec_sb[:],
                                            hivec_sb[:], op0=Alu.max, op1=Alu.min)

                ef = sampp.tile([115, HWID], f16, tag="ef")
                nc.vector.tensor_sub(ef[:], xs2[0:115], xs1[0:115])
                eb = sampp.tile([115, HWID], f16, tag="eb")
                nc.vector.tensor_sub(eb[:], xs1[0:115], xs0[0:115])
                qp = sampp.tile([115, HWID], f16, tag="qp")
                nc.vector.tensor_scalar(qp[:], q2[:], 0.0, None, op0=Alu.max)
                qm = sampp.tile([115, HWID], f16, tag="qm")
                nc.vector.tensor_scalar(qm[:], q2[:], 0.0, None, op0=Alu.min)
                t1 = sampp.tile([115, HWID], f16, tag="t1")
                nc.vector.tensor_mul(t1[:], qp[:], ef[:])
                t2 = sampp.tile([115, HWID], f16, tag="t2")
                nc.vector.tensor_mul(t2[:], qm[:], eb[:])
                s2 = sampp.tile([115, HWID], f16, tag="s2")
                nc.vector.tensor_add(s2[:], t1[:], t2[:])

                # dd[:, k, 128u+p] = D(u + 10*(128k+p)); chunk c=10k+u sits at
                # [k, 128u:128u+128]. s2/xs1 columns are chunk-pair order:
                # lower cols 128c'+p -> chunk c' (c'=0..14), upper -> c'+15.
                dd = ddp.tile([102, 3, 1280], f16, tag="dd")
                nc.vector.tensor_add(dd[0:51, 0, :], s2[0:51, 0:1280],
                                     xs1[0:51, 0:1280])
                nc.vector.tensor_add(dd[0:51, 1, 0:640], s2[0:51, 1280:1920],
                                     xs1[0:51, 1280:1920])
                nc.vector.tensor_add(dd[0:51, 1, 640:1280], s2[64:115, 0:640],
                                     xs1[64:115, 0:640])
                nc.vector.tensor_add(dd[0:51, 2, :], s2[64:115, 640:1920],
                                     xs1[64:115, 640:1920])
                # 102-stack shift: D(l+1). u<=8: chunk (k,u+1) same p -> col+128.
                # u=9: chunk (k,0) p+1 -> k-block cols 1..128.
                nc.sync.dma_start(out=dd[51:102, :, 0:1152],
                                  in_=dd[0:51, :, 128:1280])
                nc.sync.dma_start(out=dd[51:102, :, 1152:1280],
                                  in_=dd[0:51, :, 1:129])
                # u=9, p=127 wraps to the next k-block: D(10*128*(k+1))
                nc.sync.dma_start(out=dd[51:102, 0:2, 1279:1280],
                                  in_=dd[0:51, 1:3, 0:1])
                if debug and b == 0:
                    nc.sync.dma_start(out=xs1_d[:], in_=xs1[:])
                    nc.sync.dma_start(out=q2_d[:], in_=q2[:])
                    nc.sync.dma_start(out=dd_d[:], in_=dd[:])
                return dd

            def emit_final(b, dd):
                ysb = ysbp.tile([C_OUT, K, NSP], f16, tag="ysb")
                for t0 in range(K):
                    a = (STRIDE * t0) // K
                    fp = fpsum.tile([CPAD, 512], f32, tag="fp")
                    rhs = dd[0:102, :, 128 * a:128 * (a + 1)]
                    lhsT = f102_sb[0:102, t0 * CPAD:(t0 + 1) * CPAD]
                    nc.tensor.matmul(fp[:, 0:NSPP], lhsT, rhs,
                                     start=True, stop=True)
                    dst = ysb[:, t0, :]
                    src_ap = fp[0:C_OUT, 0:NSP]
                    if t0 % 2 == 1:
                        nc.vector.tensor_copy(dst, src_ap)
                    else:
                        nc.scalar.copy(dst, src_ap)
                    if t0 in (8, 16, 24, 34, 42):
                        lo = {8: 0, 16: 8, 24: 16, 34: 24, 42: 34}[t0]
                        nc.sync.dma_start(out=y_d[b, :, lo:t0],
                                          in_=ysb[:, lo:t0, :])
                nc.sync.dma_start(out=y_d[b, :, 42:K], in_=ysb[:, 42:K, :])

            emit_load(0)
            emit_load(1)
            dds = {}
            dds[0] = emit_front(0, xx_tiles[0])
            emit_load(2)
            dds[1] = emit_front(1, xx_tiles[1])
            emit_final(0, dds[0])
            emit_load(3)
            dds[2] = emit_front(2, xx_tiles[0])
            emit_final(1, dds[1])
            dds[3] = emit_front(3, xx_tiles[1])
            emit_final(2, dds[2])
            emit_final(3, dds[3])

    nc.compile()
    return nc


def _host_inputs(x, hz, band, offset_w, offset_b, B_loc=B_LOC):
    filt = _host_filters(hz, band)
    f102 = np.ascontiguousarray(_host_f102(filt).astype(np.float16))

    # wr2p rows are XS1 taps: XS1[j'] = x[10l + j']; rows 54/55 = ones
    wr2p = np.zeros((64, K), np.float32)
    wr2p[0:51, :] = offset_w[:, 0, :].T
    wr2p[54, :] = offset_b
    wr22 = np.zeros((128, 115), np.float32)
    wr22[0:64, 0:51] = wr2p
    wr22[64:128, 64:115] = wr2p
    wr22 = np.ascontiguousarray(wr22.astype(np.float16))

    ks = np.arange(K, dtype=np.float32)
    lovec = np.zeros((115, 1), np.float32)
    hivec = np.zeros((115, 1), np.float32)
    lovec[0:51, 0] = -ks
    hivec[0:51, 0] = 50 - ks
    lovec[64:115, 0] = -ks
    hivec[64:115, 0] = 50 - ks
    ident = np.ascontiguousarray(np.eye(128, dtype=np.float16))

    B = x.shape[0]
    xpad = np.zeros((B, XLEN), np.float32)
    xpad[:, 1:1 + L_FULL] = x
    p_ = np.arange(128); kk = np.arange(3); uu = np.arange(10); jj = np.arange(53)
    idx = (1 + 100 * p_[:, None, None, None] + 12800 * kk[None, :, None, None]
           + 10 * uu[None, None, :, None] + jj[None, None, None, :])
    xr = np.zeros((B, 128, 3, 10, 64), np.float32)
    xr[:, :, :, :, 0:53] = xpad[:, idx]
    xr[:, :, :, :, 54:56] = 1.0
    xr = xr.reshape(B, 128, 1920).astype(np.float16)

    n_cores = B // B_loc
    in_maps = []
    for i in range(n_cores):
        in_maps.append({
            "x": np.ascontiguousarray(xr[i * B_loc:(i + 1) * B_loc]),
            "wr22": wr22,
            "lovec": lovec,
            "hivec": hivec,
            "f102": f102,
            "ident": ident,
        })
    return in_maps


def _host_assemble(outs):
    """outs: per-core y [B_loc, C_OUT, K, NSP] f16 -> full [B, C, T_out] f32."""
    ydev = np.concatenate(outs, axis=0).astype(np.float32)  # [B, C, K, NSP]
    B = ydev.shape[0]
    y = np.empty((B, C_OUT, T_out), np.float32)
    for t0 in range(K):
        ns = (T_out - t0 + K - 1) // K
        y[:, :, t0::K] = ydev[:, :, t0, :ns]
    return y


_CACHED = {}


def _get_program():
    key = B_LOC
    if key not in _CACHED:
        _CACHED[key] = build_program(B_LOC)
    return _CACHED[key]


def kernel(x, hz, band, offset_w, offset_b):
    from concourse.bass_utils import run_bass_kernel_spmd

    x = np.asarray(x, dtype=np.float32)
    hz = np.asarray(hz, dtype=np.float32)
    band = np.asarray(band, dtype=np.float32)
    offset_w = np.asarray(offset_w, dtype=np.float32)
    offset_b = np.asarray(offset_b, dtype=np.float32)

    nc = _get_program()
    in_maps = _host_inputs(x, hz, band, offset_w, offset_b, B_LOC)
    res = run_bass_kernel_spmd(nc, in_maps, list(range(N_CORES)))
    outs = [res.results[i]["y"] for i in range(N_CORES)]
    return _host_assemble(outs)
